# revision 1
# baseline (speedup 1.0000x reference)
"""Differential attention kernel for 8 Trainium2 NeuronCores — v3.

- v1 attention structure (per-skt scores + 512-wide exp; pairing hurt).
- merged projection pass (q chunks fused, xT read once).
- per-tag psum bufs: score/proj accumulators get 4 slots, v 2, pv 4.
- weight DMAs split per d-tile, wk issued first, so the first k-projection
  matmul starts ~1us in instead of waiting 12.5us for bulk weight DMA.
"""

import math
import os
import time
from contextlib import ExitStack

import ml_dtypes
import numpy as np

import concourse.bass as bass
from concourse import bacc
import concourse.mybir as mybir
import concourse.tile as tile
from concourse.bass_utils import run_bass_kernel_spmd

B, S, D = 4, 4096, 2048
HD = 128
DV = 256
DVA = DV + 1      # + ones column for row sums
SQ = S // 2
N_CORES = 8
DEPTH = 12
SCALE = HD ** -0.5

DT_P = D // 128   # 16 d-tiles
SKT = S // 128    # 32 key tiles
SC = S // 512     # 8 s-chunks
QC = SQ // 512    # 4 q-chunks
SQT = SQ // 128   # 16 q tiles

BF16 = mybir.dt.bfloat16
F32 = mybir.dt.float32

_cache = {}


def build_nc():
    nc = bacc.Bacc("TRN2", target_bir_lowering=False, debug=False)

    xT_d = nc.declare_dram_parameter("xT", [D, S], BF16, isOutput=False)
    wq_d = nc.declare_dram_parameter("wq", [D, DV], BF16, isOutput=False)
    wk_d = nc.declare_dram_parameter("wk", [D, DV], BF16, isOutput=False)
    wv_d = nc.declare_dram_parameter("wv", [D, DV], BF16, isOutput=False)
    lam_d = nc.declare_dram_parameter("lam", [128, 1], F32, isOutput=False)
    out_d = nc.declare_dram_parameter("out", [SQ, DV], F32, isOutput=True)

    xT = xT_d.ap()
    out = out_d.ap()

    with tile.TileContext(nc) as tc, ExitStack() as ctx:
        singles = ctx.enter_context(tc.tile_pool(name="singles", bufs=1))
        x_pool = ctx.enter_context(tc.tile_pool(name="x", bufs=40))
        e_pool = ctx.enter_context(tc.tile_pool(name="e", bufs=6))
        o_pool = ctx.enter_context(tc.tile_pool(name="o", bufs=4))
        r_pool = ctx.enter_context(tc.tile_pool(name="r", bufs=8))

        # --- resident SBUF tensors; weights DMA'd per d-tile, wk first --
        w_sb = {n: singles.tile([128, DT_P, DV], BF16, tag=f"w_{n}", name=f"w_{n}")
                for n in ("wk", "wq", "wv")}
        w_ap = {"wk": wk_d.ap(), "wq": wq_d.ap(), "wv": wv_d.ap()}
        lam_sb = singles.tile([128, 1], F32, tag="lam")
        nc.sync.dma_start(out=lam_sb, in_=lam_d.ap())

        # chunk-0 x tiles interleaved with wk so the first k matmul can
        # start ~1us in; wv/wq follow (needed later in chunk 0)
        xts0 = []
        for dt in range(DT_P):
            nc.sync.dma_start(
                out=w_sb["wk"][:, dt, :],
                in_=w_ap["wk"][dt * 128:(dt + 1) * 128, :],
            )
            xt = x_pool.tile([128, 512], BF16, tag="xt", name=f"xt0_{dt}")
            nc.sync.dma_start(out=xt, in_=xT[dt * 128:(dt + 1) * 128, 0:512])
            xts0.append(xt)
        for n in ("wv", "wq"):
            for dt in range(DT_P):
                nc.sync.dma_start(
                    out=w_sb[n][:, dt, :],
                    in_=w_ap[n][dt * 128:(dt + 1) * 128, :],
                )

        kT = singles.tile([128, 2, S], BF16, tag="kT")        # [dh, head, sk]
        qT = singles.tile([128, 2, SQ], BF16, tag="qT")       # [dh, head, sq]
        v_aug = singles.tile([128, SKT, DVA], BF16, tag="v")  # [s_row, s_tile, dv+1]
        pv1 = singles.tile([128, SQT, DVA], F32, tag="pv1")   # head-1 PV staging

        nc.vector.memset(v_aug[:, :, DV:DVA], 1.0)            # ones column

        # --- projections: one pass over the 8 s-chunks ------------------
        pctx = ExitStack()
        psum = pctx.enter_context(
            tc.tile_pool(name="psum_proj", bufs=4, space=bass.MemorySpace.PSUM)
        )

        # PE warm-up: junk matmuls fill the initial DMA wait so HAM is at
        # 2.4GHz when the first real projection matmul issues.
        jt = singles.tile([128, 512], BF16, tag="junk")
        nc.vector.memset(jt, 0.0)
        jps = psum.tile([128, 512], F32, tag="big_ps", bufs=4, name="jps")
        for w in range(48):
            nc.tensor.matmul(jps, jt[:, 0:128], jt, start=True, stop=True)
        nc.vector.tensor_copy(jt, jps)
        for sc in range(SC):
            if sc == 0:
                xts = xts0
            else:
                xts = []
                for dt in range(DT_P):
                    xt = x_pool.tile([128, 512], BF16, tag="xt", name=f"xt{sc}_{dt}")
                    nc.sync.dma_start(
                        out=xt,
                        in_=xT[dt * 128:(dt + 1) * 128, sc * 512:(sc + 1) * 512],
                    )
                    xts.append(xt)

            projs = [("wk", kT)] + ([("wq", qT)] if sc < QC else [])
            for wname, dst in projs:
                for h in range(2):
                    ps = psum.tile([128, 512], F32, tag="big_ps", bufs=4,
                                   name=f"ps{sc}{wname}{h}")
                    for dt in range(DT_P):
                        nc.tensor.matmul(
                            ps,
                            w_sb[wname][:, dt, h * HD:(h + 1) * HD],
                            xts[dt],
                            start=(dt == 0),
                            stop=(dt == DT_P - 1),
                        )
                    nc.vector.tensor_copy(dst[:, h, sc * 512:(sc + 1) * 512], ps)
            for i in range(4):
                vps = psum.tile([128, DV], F32, tag="v_ps", bufs=2,
                                name=f"vps{sc}_{i}")
                for dt in range(DT_P):
                    nc.tensor.matmul(
                        vps,
                        xts[dt][:, i * 128:(i + 1) * 128],
                        w_sb["wv"][:, dt, :],
                        start=(dt == 0),
                        stop=(dt == DT_P - 1),
                    )
                nc.vector.tensor_copy(v_aug[:, sc * 4 + i, 0:DV], vps)

        pctx.close()

        # --- attention: head 1 then head 2 ------------------------------
        psum = ctx.enter_context(
            tc.tile_pool(name="psum_att", bufs=4, space=bass.MemorySpace.PSUM)
        )
        psum_pv = ctx.enter_context(
            tc.tile_pool(name="psum_pv", bufs=4, space=bass.MemorySpace.PSUM)
        )
        for h in range(2):
            for qc in range(QC):
                pv_ps = [
                    psum_pv.tile([128, DVA], F32, tag="pv_ps", name=f"pv_ps{i}")
                    for i in range(4)
                ]
                for skt in range(SKT):
                    sps = psum.tile([128, 512], F32, tag="big_ps", bufs=4,
                                    name=f"sps{skt}")
                    nc.tensor.matmul(
                        sps,
                        kT[:, h, skt * 128:(skt + 1) * 128],
                        qT[:, h, qc * 512:(qc + 1) * 512],
                    )
                    et = e_pool.tile([128, 512], BF16, tag="et", name=f"et{skt}")
                    nc.scalar.activation(
                        out=et, in_=sps,
                        func=mybir.ActivationFunctionType.Exp,
                        scale=SCALE,
                    )
                    for i in range(4):
                        nc.tensor.matmul(
                            pv_ps[i],
                            et[:, i * 128:(i + 1) * 128],
                            v_aug[:, skt, :],
                            start=(skt == 0),
                            stop=(skt == SKT - 1),
                        )
                for i in range(4):
                    idx = qc * 4 + i
                    if h == 0:
                        nc.vector.tensor_copy(pv1[:, idx, :], pv_ps[i])
                    else:
                        r1 = r_pool.tile([128, 1], F32, tag="r1", name=f"r1_{idx}")
                        r2 = r_pool.tile([128, 1], F32, tag="r2", name=f"r2_{idx}")
                        nc.vector.reciprocal(r1, pv1[:, idx, DV:DVA])
                        nc.vector.reciprocal(r2, pv_ps[i][:, DV:DVA])
                        r2l = r_pool.tile([128, 1], F32, tag="r2l", name=f"r2l_{idx}")
                        nc.vector.tensor_mul(r2l, r2, lam_sb)
                        o1 = o_pool.tile([128, DV], F32, tag="o1", name=f"o1_{idx}")
                        o2 = o_pool.tile([128, DV], F32, tag="o2", name=f"o2_{idx}")
                        nc.vector.tensor_scalar_mul(o1, pv1[:, idx, 0:DV], r1)
                        nc.vector.tensor_scalar_mul(o2, pv_ps[i][:, 0:DV], r2l)
                        ot = o_pool.tile([128, DV], F32, tag="ot", name=f"ot_{idx}")
                        nc.vector.tensor_sub(ot, o1, o2)
                        nc.sync.dma_start(
                            out=out[idx * 128:(idx + 1) * 128, :], in_=ot
                        )

    nc.compile()
    return nc


def _lam(lambda_q1, lambda_q2, lambda_k1, lambda_k2):
    lam_init = 0.8 - 0.6 * math.exp(-0.3 * DEPTH)
    l1 = math.exp(float(np.sum(lambda_q1.astype(np.float64) * lambda_k1.astype(np.float64))))
    l2 = math.exp(float(np.sum(lambda_q2.astype(np.float64) * lambda_k2.astype(np.float64))))
    return l1 + l2 + lam_init


def kernel(x, WQ, WK, WV, lambda_q1, lambda_q2, lambda_k1, lambda_k2):
    if "nc" not in _cache:
        _cache["nc"] = build_nc()
    nc = _cache["nc"]

    bf = ml_dtypes.bfloat16
    lam = np.full((128, 1), _lam(lambda_q1, lambda_q2, lambda_k1, lambda_k2), np.float32)
    wq = np.ascontiguousarray(WQ, dtype=bf)
    wk = np.ascontiguousarray(WK, dtype=bf)
    wv = np.ascontiguousarray(WV, dtype=bf)

    in_maps = []
    for c in range(N_CORES):
        b, qs = c // 2, (c % 2) * SQ
        xb = x[b] if qs == 0 else np.concatenate([x[b, qs:], x[b, :qs]], axis=0)
        xT = np.ascontiguousarray(xb.T, dtype=bf)
        in_maps.append({"xT": xT, "wq": wq, "wk": wk, "wv": wv, "lam": lam})

    kres = None
    for attempt in range(3):
        try:
            kres = run_bass_kernel_spmd(nc, in_maps, list(range(N_CORES)))
            break
        except (ModuleNotFoundError, ImportError):
            # BASS_TRACE requested but this axon build has no NTFF hook
            os.environ["BASS_NEVER_TRACE"] = "1"
        except Exception:
            if attempt == 2:
                raise
            time.sleep(5)
    if kres is None:
        kres = run_bass_kernel_spmd(nc, in_maps, list(range(N_CORES)))
    _cache["last_results"] = kres
    res = kres.results

    out = np.empty((B, S, DV), np.float32)
    for c in range(N_CORES):
        b, qs = c // 2, (c % 2) * SQ
        out[b, qs:qs + SQ] = res[c]["out"]
    return out



# revision 40
# speedup vs baseline: 1.1093x; 1.1093x over previous
"""Differential attention kernel for 8 Trainium2 NeuronCores — v3.

Structure vs the v1 two-PV baseline:
- fp16 everywhere instead of bf16 (same cost-model speed, 4x better accuracy).
- single combined PV: attn = e1 - m_b*e2 with m = lam*s1/s2 per query,
  halving the PV matmul cycles (110us -> 55us of PE).
- row sums via N=1 matmuls (cost model charges only the output free dim),
  sharing one PSUM bank across all 8 accumulation chains (shared start).
- m broadcast to a [128, 512] matrix via DMA sbuf->sbuf transpose of the
  per-qslice [128,1] columns plus a K=1 PE matmul with a ones row.
- qc0's scores+exp interleaved into the projection phase so the scalar
  engine's 16.7M-element exp workload starts ~20us in instead of ~140us.
- PV accumulates 4 q-slices in 2 PSUM banks (2 chains per bank, shared
  start/stop) in a single pass per key tile.
"""

import math
import os
import time
from contextlib import ExitStack

import ml_dtypes
import numpy as np

import concourse.bass as bass
from concourse import bacc
import concourse.mybir as mybir
import concourse.tile as tile
from concourse.bass_utils import run_bass_kernel_spmd

B, S, D = 4, 4096, 2048
HD = 128
DV = 256
SQ = S // 2
N_CORES = 8
DEPTH = 12
SCALE = HD ** -0.5
EXP_BIAS = -2.0

DT_P = D // 128   # 16 d-tiles
SKT = S // 128    # 32 key tiles
SC = S // 512     # 8 s-chunks
QC = SQ // 512    # 4 q-chunks
SQT = SQ // 128   # 16 q tiles

F16 = mybir.dt.float16
F32 = mybir.dt.float32

_cache = {}


def build_nc():
    nc = bacc.Bacc("TRN2", target_bir_lowering=False, debug=False)

    xT_d = nc.declare_dram_parameter("xT", [D, S], F16, isOutput=False)
    wq_d = nc.declare_dram_parameter("wq", [D, DV], F16, isOutput=False)
    wk_d = nc.declare_dram_parameter("wk", [D, DV], F16, isOutput=False)
    wv_d = nc.declare_dram_parameter("wv", [D, DV], F16, isOutput=False)
    lam_d = nc.declare_dram_parameter("lam", [128, 1], F32, isOutput=False)
    out_d = nc.declare_dram_parameter("out", [SQ, DV], F32, isOutput=True)

    xT = xT_d.ap()
    out = out_d.ap()
    Exp = mybir.ActivationFunctionType.Exp

    with tile.TileContext(nc) as tc, ExitStack() as ctx:
        singles = ctx.enter_context(tc.tile_pool(name="singles", bufs=1))
        o_pool = ctx.enter_context(tc.tile_pool(name="o", bufs=3))
        r_pool = ctx.enter_context(tc.tile_pool(name="r", bufs=10))
        u_pool = ctx.enter_context(tc.tile_pool(name="u", bufs=3))
        mb_pool = ctx.enter_context(tc.tile_pool(name="mb", bufs=2))
        mrow_pool = ctx.enter_context(tc.tile_pool(name="mrow", bufs=2))
        # et pool A (full 64KB/partition tiles) serves qc0/qc2 as a ring of
        # one; qc1/qc3 are split lo/hi.  Pool B-lo must open before the
        # proj-phase pools (stack allocator) since it outlives them; B-hi
        # opens after they pop.
        et_a = ctx.enter_context(tc.tile_pool(name="et_a", bufs=1))
        et_b_lo = ctx.enter_context(tc.tile_pool(name="et_b_lo", bufs=1))

        psum_aux = ctx.enter_context(
            tc.tile_pool(name="psum_aux", bufs=2, space=bass.MemorySpace.PSUM)
        )

        # --- persistent SBUF tensors ------------------------------------
        kT = singles.tile([128, 2, S], F16, tag="kT")        # [dh, head, key]
        qT = singles.tile([128, 2, SQ], F16, tag="qT")       # [dh, head, q]
        v_sb = singles.tile([128, SKT, DV], F16, tag="v")    # [krow, ktile, dv]
        lam_sb = singles.tile([128, 1], F32, tag="lam")
        ones_col = singles.tile([128, 1], F16, tag="ones_col")
        ones_row = singles.tile([1, 128], F16, tag="ones_row")
        jt = singles.tile([128, 512], F16, tag="junk")

        bias_sb = singles.tile([128, 1], F32, tag="bias_sb")
        nc.sync.dma_start(out=lam_sb, in_=lam_d.ap())
        nc.vector.memset(ones_col, 1.0)
        nc.vector.memset(ones_row, 1.0)
        nc.vector.memset(jt, 0.0)
        nc.vector.memset(bias_sb, EXP_BIAS)

        proj_ctx = ExitStack()
        w_pool = proj_ctx.enter_context(tc.tile_pool(name="w", bufs=1))
        x_pool = proj_ctx.enter_context(tc.tile_pool(name="x", bufs=2))
        psum_proj = proj_ctx.enter_context(
            tc.tile_pool(name="psum_proj", bufs=2, space=bass.MemorySpace.PSUM)
        )
        psum_v = proj_ctx.enter_context(
            tc.tile_pool(name="psum_v", bufs=2, space=bass.MemorySpace.PSUM)
        )
        psum_s1 = proj_ctx.enter_context(
            tc.tile_pool(name="psum_s1", bufs=2, space=bass.MemorySpace.PSUM)
        )

        # x chunks: [128, dt, 512] layout; chunk 0 is split into quarters
        # so the first K-projection matmuls can start as soon as wk plus a
        # quarter of x0 have landed.
        def load_x_chunk(sc, split=1):
            xt = x_pool.tile([128, DT_P, 512], F16, tag="xt", name=f"x{sc}")
            q = DT_P // split
            for i in range(split):
                nc.sync.dma_start(
                    out=xt[:, i * q:(i + 1) * q, :],
                    in_=xT[i * q * 128:(i + 1) * q * 128,
                           sc * 512:(sc + 1) * 512].rearrange(
                        "(t p) v -> p t v", p=128
                    ),
                )
            return xt

        # DMA order: wk, x0 (split), wv, x1, wq — matching first use.  wq
        # lives in its own pool, closed after the Q projection finishes so
        # qc1's lo et tile can take its SBUF.
        wq_ctx = ExitStack()
        wq_pool = wq_ctx.enter_context(tc.tile_pool(name="wqp", bufs=1))
        w_sb = {}

        def load_w(name, dram, pool):
            w_sb[name] = pool.tile([128, DT_P, DV], F16, tag=f"w_{name}",
                                   name=f"w_{name}")
            nc.sync.dma_start(
                out=w_sb[name],
                in_=dram.ap().rearrange("(t p) v -> p t v", p=128),
            )

        load_w("wk", wk_d, w_pool)
        xt0 = load_x_chunk(0, split=4)
        load_w("wv", wv_d, w_pool)
        xt1 = load_x_chunk(1)
        load_w("wq", wq_d, wq_pool)

        # PE warm-up: junk matmuls cover the initial DMA wait and the
        # p-state ramp; a junk exp pulls the ACT table load off the
        # critical path.
        jps = psum_proj.tile([128, 512], F32, tag="big", name="jps")
        for _ in range(14):
            nc.tensor.matmul(jps, jt[:, 0:128], jt, start=True, stop=True)
        jet = r_pool.tile([128, 1], F16, tag="jet", name="jet")
        nc.scalar.activation(out=jet, in_=jt[:, 0:1], func=Exp, scale=SCALE)

        # et tensors: [128, head, kts, 512] fp16.  qc0/qc2 are single
        # 64KB/partition tiles; qc1/qc3 are split into lo/hi 16-kt halves.
        et_parts = {}

        def et_alloc_full(pool, qc):
            t = pool.tile([128, 2, SKT, 512], F16, tag="et", name=f"et_{qc}")
            et_parts[qc] = [(t, 0)]

        def et_alloc_half(pool, qc, half):
            t = pool.tile([128, 2, SKT // 2, 512], F16, tag="eth",
                          name=f"et_{qc}_{half}")
            et_parts.setdefault(qc, []).append((t, half * (SKT // 2)))

        def e_part(qc, kt):
            for t, base in et_parts[qc]:
                if base <= kt < base + (SKT if len(et_parts[qc]) == 1 else SKT // 2):
                    return t, kt - base
            raise KeyError((qc, kt))

        def e_slab(qc, h, kt):
            t, k = e_part(qc, kt)
            return t[:, h, k, :]

        def e_both(qc, kt):
            t, k = e_part(qc, kt)
            return t[:, :, k, :]

        et_alloc_full(et_a, 0)

        # aux psum: sums for qc in columns 0..7 (h*4+qsl), plus the m
        # broadcast matmuls reuse the same pool.
        sums_ps = {}
        sums_started = {}

        def sum_mms(qc, kt):
            """N=1 matmuls accumulating exp row-sums; issue after scores kt+1."""
            aux = sums_ps[qc]
            for h in range(2):
                e = e_slab(qc, h, kt)
                for qsl in range(4):
                    first = not sums_started.get(qc)
                    sums_started[qc] = True
                    last = kt == SKT - 1 and h == 1 and qsl == 3
                    nc.tensor.matmul(
                        aux[:, h * 4 + qsl: h * 4 + qsl + 1],
                        e[:, qsl * 128:(qsl + 1) * 128],
                        ones_col,
                        start=first,
                        stop=last,
                        skip_group_check=not (first or last),
                    )

        # --- projection phase, with qc0 scores/exp interleaved ----------
        sps1 = {}
        for h in range(2):
            sps1[h] = psum_s1.tile([128, 512], F32, tag="sps1", name=f"sps1_{h}")
        sums_ps[0] = psum_aux.tile([128, 512], F32, tag="aux", name="aux0")

        # Scores for qc0 (all chunks) and qc1's lo half are woven between
        # the projection MM groups so each exp has slack before the PE
        # needs its psum tile back (single-wide, one tile per head).
        _SENT = object()
        stepq = []

        def score_steps(qc, kts, qcol):
            """Generator: one (score MM + exp + lagged sums) per next()."""
            for kt in kts:
                for h in range(2):
                    nc.tensor.matmul(
                        sps1[h], kT[:, h, kt * 128:(kt + 1) * 128],
                        qT[:, h, qcol:qcol + 512], start=True, stop=True,
                    )
                    nc.scalar.activation(
                        out=e_slab(qc, h, kt), in_=sps1[h],
                        func=Exp, scale=SCALE, bias=bias_sb,
                    )
                if kt >= 2:
                    sum_mms(qc, kt - 2)
                yield

        def step(n=1):
            for _ in range(n):
                while stepq:
                    if next(stepq[0], _SENT) is _SENT:
                        stepq.pop(0)
                        continue
                    break

        xts = [xt0, xt1]
        for sc in range(SC):
            xt = xts[sc % 2]
            if sc == QC:
                wq_ctx.close()
                et_alloc_half(et_b_lo, 1, 0)
                sums_ps[1] = psum_aux.tile([128, 512], F32, tag="aux",
                                           name="aux1")
            if sc > 0:
                stepq.append(score_steps(0, range((sc - 1) * 4, sc * 4), 0))
            if sc >= QC:
                p = sc - QC
                stepq.append(score_steps(1, range(p * 4, p * 4 + 4), 512))
            # K projection (both heads)
            for h in range(2):
                ps = psum_proj.tile([128, 512], F32, tag="big", name=f"k{sc}{h}")
                for dt in range(DT_P):
                    nc.tensor.matmul(
                        ps,
                        w_sb["wk"][:, dt, h * HD:(h + 1) * HD],
                        xt[:, dt, :],
                        start=(dt == 0),
                        stop=(dt == DT_P - 1),
                    )
                nc.vector.tensor_copy(kT[:, h, sc * 512:(sc + 1) * 512], ps)
                step()
            # Q projection (first 4 chunks only)
            if sc < QC:
                for h in range(2):
                    ps = psum_proj.tile([128, 512], F32, tag="big", name=f"q{sc}{h}")
                    for dt in range(DT_P):
                        nc.tensor.matmul(
                            ps,
                            w_sb["wq"][:, dt, h * HD:(h + 1) * HD],
                            xt[:, dt, :],
                            start=(dt == 0),
                            stop=(dt == DT_P - 1),
                        )
                    nc.vector.tensor_copy(qT[:, h, sc * 512:(sc + 1) * 512], ps)
                    step()
            # V projection (4 s-tiles per chunk)
            for i in range(4):
                vps = psum_v.tile([128, DV], F32, tag="vps", name=f"v{sc}_{i}")
                for dt in range(DT_P):
                    nc.tensor.matmul(
                        vps,
                        xt[:, dt, i * 128:(i + 1) * 128],
                        w_sb["wv"][:, dt, :],
                        start=(dt == 0),
                        stop=(dt == DT_P - 1),
                    )
                nc.vector.tensor_copy(v_sb[:, sc * 4 + i, :], vps)
                step()
            # prefetch chunk sc+2
            if sc + 2 < SC:
                xts[sc % 2] = load_x_chunk(sc + 2)
            # drain this chunk's pending score steps
            step(8)
        # chunk 7's qc0 scores + trailing sums
        stepq.append(score_steps(0, range((SC - 1) * 4, SC * 4), 0))
        step(9)
        for kt in (SKT - 2, SKT - 1):
            sum_mms(0, kt)

        proj_ctx.close()

        # --- attention phase --------------------------------------------
        et_b_hi = ctx.enter_context(tc.tile_pool(name="et_b_hi", bufs=1))
        et_alloc_half(et_b_hi, 1, 1)
        psum_s2 = ctx.enter_context(
            tc.tile_pool(name="psum_s2", bufs=2, space=bass.MemorySpace.PSUM)
        )
        psum_pv = ctx.enter_context(
            tc.tile_pool(name="psum_pv", bufs=2, space=bass.MemorySpace.PSUM)
        )

        def pv_kts(qc, kts, pv):
            for kt in kts:
                for qsl in range(4):
                    first = kt == 0 and qsl % 2 == 0
                    last = kt == SKT - 1 and qsl % 2 == 1
                    t, k = e_part(qc, kt)
                    nc.tensor.matmul(
                        pv[qsl // 2][:, (qsl % 2) * 256:(qsl % 2) * 256 + 256],
                        t[:, 0, k, qsl * 128:(qsl + 1) * 128],
                        v_sb[:, kt, :],
                        start=first,
                        stop=last,
                        skip_group_check=not (first or last),
                    )

        def weave_qc(qc, prev, kt_start=0):
            """2-wide scores+exp for qc's kts [kt_start..32), woven with
            sums(qc) and the whole PV(prev).

            Per-iteration PE order is [pv][sums][scores] so independent work
            precedes the ACT-gated score matmuls; the previous qc's
            m-broadcast + combine issue after iteration 0 (by then the
            m-chain recips/DMA have had a scores-iteration to run).
            """
            if qc not in sums_ps:
                sums_ps[qc] = psum_aux.tile([128, 512], F32, tag="aux",
                                            name=f"aux{qc}")
            if qc == 2:
                et_alloc_full(et_a, 2)
            elif qc == 3:
                et_alloc_half(et_b_lo, 3, 0)
                et_alloc_half(et_b_hi, 3, 1)
            pv = [
                psum_pv.tile([128, 512], F32, tag="pv", name=f"pv{prev}_{b}")
                for b in range(2)
            ]
            n_it = SKT - kt_start
            pv_per = 2 if n_it <= 16 else 1
            for i, kt in enumerate(range(kt_start, SKT)):
                if i == 1:
                    prep_pe(prev)
                    combine_qc(prev)
                if i >= 2:
                    base = (i - 2) * pv_per
                    pv_kts(prev, range(base, min(base + pv_per, SKT - 4)), pv)
                if kt >= 2:
                    sum_mms(qc, kt - 2)
                sps = psum_s2.tile([128, 2, 512], F32, tag="sps2", bufs=2,
                                   name=f"s{qc}_{kt}")
                for h in range(2):
                    nc.tensor.matmul(
                        sps[:, h, :],
                        kT[:, h, kt * 128:(kt + 1) * 128],
                        qT[:, h, qc * 512:(qc + 1) * 512],
                        start=True,
                        stop=True,
                    )
                nc.scalar.activation(
                    out=e_both(qc, kt),
                    in_=sps,
                    func=Exp, scale=SCALE, bias=bias_sb,
                )
            pv_kts(prev, (SKT - 4, SKT - 3, SKT - 2, SKT - 1), pv)
            for kt in (SKT - 2, SKT - 1):
                sum_mms(qc, kt)
            finish_qc(prev, pv)

        def finish_qc(qc, pv):
            r1s = prep_state[qc]["r1s"]
            for qsl in range(4):
                idx = qc * 4 + qsl
                ot = o_pool.tile([128, DV], F32, tag="ot", name=f"ot{idx}")
                nc.vector.tensor_scalar_mul(
                    ot, pv[qsl // 2][:, (qsl % 2) * 256:(qsl % 2) * 256 + 256],
                    r1s[qsl],
                )
                nc.sync.dma_start(out=out[idx * 128:(idx + 1) * 128, :], in_=ot)

        prep_state = {}

        def prep_ve(qc):
            """m-chain VE part: reciprocals, m column, one transpose DMA."""
            aux = sums_ps[qc]
            m_row = mrow_pool.tile([1, 512], F16, tag="mrow", name=f"mrow{qc}")
            m4 = r_pool.tile([128, 4], F16, tag="m4", name=f"m4_{qc}")
            r1s = []
            for qsl in range(4):
                r1 = r_pool.tile([128, 1], F32, tag="r1", name=f"r1_{qc}_{qsl}")
                nc.vector.reciprocal(r1, aux[:, qsl:qsl + 1])
                r1s.append(r1)
                rs2 = r_pool.tile([128, 1], F32, tag="rs2", name=f"rs2_{qc}_{qsl}")
                nc.vector.reciprocal(rs2, aux[:, 4 + qsl:5 + qsl])
                mt = r_pool.tile([128, 1], F32, tag="mt", name=f"mt_{qc}_{qsl}")
                nc.vector.tensor_mul(mt, aux[:, qsl:qsl + 1], rs2)
                nc.vector.tensor_mul(m4[:, qsl:qsl + 1], mt, lam_sb)
            for qsl in range(4):
                nc.sync.dma_start(
                    out=m_row[0:1, qsl * 128:(qsl + 1) * 128],
                    in_=m4[:, qsl:qsl + 1],
                )
            prep_state[qc] = {"r1s": r1s, "m_row": m_row}

        def prep_pe(qc):
            """m broadcast: K=1 matmul + copy to SBUF fp16."""
            mb_ps = psum_aux.tile([128, 512], F32, tag="aux", name=f"mb_ps{qc}")
            nc.tensor.matmul(mb_ps, ones_row, prep_state[qc]["m_row"],
                             start=True, stop=True)
            m_b = mb_pool.tile([128, 512], F16, tag="mb", name=f"mb{qc}")
            nc.vector.tensor_copy(m_b, mb_ps)
            prep_state[qc]["m_b"] = m_b

        def combine_qc(qc):
            """VE/Pool combine, writing attn in place over the e1 slab."""
            m_b = prep_state[qc]["m_b"]
            for kt in range(SKT):
                u = u_pool.tile([128, 512], F16, tag="u", name=f"u{qc}_{kt}")
                eng = nc.gpsimd if kt % 3 == 2 else nc.vector
                eng.tensor_mul(u, e_slab(qc, 1, kt), m_b)
                nc.vector.tensor_sub(e_slab(qc, 0, kt), e_slab(qc, 0, kt), u)

        prep_ve(0)
        weave_qc(1, prev=0, kt_start=SKT // 2)
        prep_ve(1)
        weave_qc(2, prev=1)
        prep_ve(2)
        weave_qc(3, prev=2)
        prep_ve(3)
        prep_pe(3)
        combine_qc(3)
        # drain: PV for the last qc
        pv3 = [
            psum_pv.tile([128, 512], F32, tag="pv", name=f"pv3_{b}")
            for b in range(2)
        ]
        for g in range(SKT // 2):
            pv_kts(3, (2 * g, 2 * g + 1), pv3)
        finish_qc(3, pv3)

    nc.compile()
    return nc


def _lam(lambda_q1, lambda_q2, lambda_k1, lambda_k2):
    lam_init = 0.8 - 0.6 * math.exp(-0.3 * DEPTH)
    l1 = math.exp(float(np.sum(lambda_q1.astype(np.float64) * lambda_k1.astype(np.float64))))
    l2 = math.exp(float(np.sum(lambda_q2.astype(np.float64) * lambda_k2.astype(np.float64))))
    return l1 + l2 + lam_init


def kernel(x, WQ, WK, WV, lambda_q1, lambda_q2, lambda_k1, lambda_k2):
    if "nc" not in _cache:
        _cache["nc"] = build_nc()
    nc = _cache["nc"]

    lam = np.full((128, 1), _lam(lambda_q1, lambda_q2, lambda_k1, lambda_k2), np.float32)
    wq = np.ascontiguousarray(WQ, dtype=np.float16)
    wk = np.ascontiguousarray(WK, dtype=np.float16)
    wv = np.ascontiguousarray(WV, dtype=np.float16)

    in_maps = []
    for c in range(N_CORES):
        b, qs = c // 2, (c % 2) * SQ
        xb = x[b] if qs == 0 else np.concatenate([x[b, qs:], x[b, :qs]], axis=0)
        xTc = np.ascontiguousarray(xb.T, dtype=np.float16)
        in_maps.append({"xT": xTc, "wq": wq, "wk": wk, "wv": wv, "lam": lam})

    kres = None
    for attempt in range(3):
        try:
            kres = run_bass_kernel_spmd(nc, in_maps, list(range(N_CORES)))
            break
        except (ModuleNotFoundError, ImportError):
            os.environ["BASS_NEVER_TRACE"] = "1"
        except Exception:
            if attempt == 2:
                raise
            time.sleep(5)
    if kres is None:
        kres = run_bass_kernel_spmd(nc, in_maps, list(range(N_CORES)))
    _cache["last_results"] = kres
    res = kres.results

    out = np.empty((B, S, DV), np.float32)
    for c in range(N_CORES):
        b, qs = c // 2, (c % 2) * SQ
        out[b, qs:qs + SQ] = res[c]["out"]
    return out


# revision 46
# speedup vs baseline: 1.1152x; 1.0052x over previous
"""Differential attention kernel for 8 Trainium2 NeuronCores — v3.

Structure vs the v1 two-PV baseline:
- fp16 everywhere instead of bf16 (same cost-model speed, 4x better accuracy).
- single combined PV: attn = e1 - m_b*e2 with m = lam*s1/s2 per query,
  halving the PV matmul cycles (110us -> 55us of PE).
- row sums via N=1 matmuls (cost model charges only the output free dim),
  sharing one PSUM bank across all 8 accumulation chains (shared start).
- m broadcast to a [128, 512] matrix via DMA sbuf->sbuf transpose of the
  per-qslice [128,1] columns plus a K=1 PE matmul with a ones row.
- qc0's scores+exp interleaved into the projection phase so the scalar
  engine's 16.7M-element exp workload starts ~20us in instead of ~140us.
- PV accumulates 4 q-slices in 2 PSUM banks (2 chains per bank, shared
  start/stop) in a single pass per key tile.
"""

import math
import os
import time
from contextlib import ExitStack

import ml_dtypes
import numpy as np

import concourse.bass as bass
from concourse import bacc
import concourse.mybir as mybir
import concourse.tile as tile
from concourse.bass_utils import run_bass_kernel_spmd

B, S, D = 4, 4096, 2048
HD = 128
DV = 256
SQ = S // 2
N_CORES = 8
DEPTH = 12
SCALE = HD ** -0.5
EXP_BIAS = -2.0

DT_P = D // 128   # 16 d-tiles
SKT = S // 128    # 32 key tiles
SC = S // 512     # 8 s-chunks
QC = SQ // 512    # 4 q-chunks
SQT = SQ // 128   # 16 q tiles

F16 = mybir.dt.float16
F32 = mybir.dt.float32

_cache = {}


def build_nc():
    nc = bacc.Bacc("TRN2", target_bir_lowering=False, debug=False)

    xT_d = nc.declare_dram_parameter("xT", [D, S], F16, isOutput=False)
    wq_d = nc.declare_dram_parameter("wq", [D, DV], F16, isOutput=False)
    wk_d = nc.declare_dram_parameter("wk", [D, DV], F16, isOutput=False)
    wv_d = nc.declare_dram_parameter("wv", [D, DV], F16, isOutput=False)
    lam_d = nc.declare_dram_parameter("lam", [128, 1], F32, isOutput=False)
    out_d = nc.declare_dram_parameter("out", [SQ, DV], F32, isOutput=True)

    xT = xT_d.ap()
    out = out_d.ap()
    Exp = mybir.ActivationFunctionType.Exp

    with tile.TileContext(nc) as tc, ExitStack() as ctx:
        singles = ctx.enter_context(tc.tile_pool(name="singles", bufs=1))
        o_pool = ctx.enter_context(tc.tile_pool(name="o", bufs=3))
        r_pool = ctx.enter_context(tc.tile_pool(name="r", bufs=10))
        u_pool = ctx.enter_context(tc.tile_pool(name="u", bufs=3))
        mb_pool = ctx.enter_context(tc.tile_pool(name="mb", bufs=2))
        mrow_pool = ctx.enter_context(tc.tile_pool(name="mrow", bufs=2))
        # et pool A (full 64KB/partition tiles) serves qc0/qc2 as a ring of
        # one; qc1/qc3 are split lo/hi.  Pool B-lo must open before the
        # proj-phase pools (stack allocator) since it outlives them; B-hi
        # opens after they pop.
        et_a = ctx.enter_context(tc.tile_pool(name="et_a", bufs=1))
        et_b_lo = ctx.enter_context(tc.tile_pool(name="et_b_lo", bufs=1))

        psum_aux = ctx.enter_context(
            tc.tile_pool(name="psum_aux", bufs=2, space=bass.MemorySpace.PSUM)
        )

        # --- persistent SBUF tensors ------------------------------------
        kT = singles.tile([128, 2, S], F16, tag="kT")        # [dh, head, key]
        qT = singles.tile([128, 2, SQ], F16, tag="qT")       # [dh, head, q]
        v_sb = singles.tile([128, SKT, DV], F16, tag="v")    # [krow, ktile, dv]
        lam_sb = singles.tile([128, 1], F32, tag="lam")
        ones_col = singles.tile([128, 1], F16, tag="ones_col")
        ones_row = singles.tile([1, 128], F16, tag="ones_row")
        jt = singles.tile([128, 512], F16, tag="junk")

        bias_sb = singles.tile([128, 1], F32, tag="bias_sb")
        nc.sync.dma_start(out=lam_sb, in_=lam_d.ap())
        nc.vector.memset(ones_col, 1.0)
        nc.vector.memset(ones_row, 1.0)
        nc.vector.memset(jt, 0.0)
        nc.vector.memset(bias_sb, EXP_BIAS)

        proj_ctx = ExitStack()
        w_pool = proj_ctx.enter_context(tc.tile_pool(name="w", bufs=1))
        x_pool = proj_ctx.enter_context(tc.tile_pool(name="x", bufs=2))
        psum_proj = proj_ctx.enter_context(
            tc.tile_pool(name="psum_proj", bufs=2, space=bass.MemorySpace.PSUM)
        )
        psum_v = proj_ctx.enter_context(
            tc.tile_pool(name="psum_v", bufs=2, space=bass.MemorySpace.PSUM)
        )
        psum_s1 = proj_ctx.enter_context(
            tc.tile_pool(name="psum_s1", bufs=2, space=bass.MemorySpace.PSUM)
        )

        # x chunks: [128, dt, 512] layout; chunk 0 is split into quarters
        # so the first K-projection matmuls can start as soon as wk plus a
        # quarter of x0 have landed.
        def load_x_chunk(sc, split=1):
            xt = x_pool.tile([128, DT_P, 512], F16, tag="xt", name=f"x{sc}")
            q = DT_P // split
            for i in range(split):
                nc.sync.dma_start(
                    out=xt[:, i * q:(i + 1) * q, :],
                    in_=xT[i * q * 128:(i + 1) * q * 128,
                           sc * 512:(sc + 1) * 512].rearrange(
                        "(t p) v -> p t v", p=128
                    ),
                )
            return xt

        # DMA order: wk, x0 (split), wv, x1, wq — matching first use.  wq
        # lives in its own pool, closed after the Q projection finishes so
        # qc1's lo et tile can take its SBUF.
        wq_ctx = ExitStack()
        wq_pool = wq_ctx.enter_context(tc.tile_pool(name="wqp", bufs=1))
        w_sb = {}

        def load_w(name, dram, pool, split=1):
            t = pool.tile([128, DT_P, DV], F16, tag=f"w_{name}",
                          name=f"w_{name}")
            w_sb[name] = t
            q = DT_P // split
            for i in range(split):
                nc.sync.dma_start(
                    out=t[:, i * q:(i + 1) * q, :],
                    in_=dram.ap()[i * q * 128:(i + 1) * q * 128, :].rearrange(
                        "(t p) v -> p t v", p=128
                    ),
                )

        load_w("wk", wk_d, w_pool, split=2)
        xt0 = load_x_chunk(0, split=4)
        load_w("wv", wv_d, w_pool)
        xt1 = load_x_chunk(1)
        load_w("wq", wq_d, wq_pool)

        # PE warm-up: junk matmuls cover the initial DMA wait and the
        # p-state ramp; a junk exp pulls the ACT table load off the
        # critical path.
        jps = psum_proj.tile([128, 512], F32, tag="big", name="jps")
        for _ in range(12):
            nc.tensor.matmul(jps, jt[:, 0:128], jt, start=True, stop=True)
        jet = r_pool.tile([128, 1], F16, tag="jet", name="jet")
        nc.scalar.activation(out=jet, in_=jt[:, 0:1], func=Exp, scale=SCALE)

        # et tensors: [128, head, kts, 512] fp16.  qc0/qc2 are single
        # 64KB/partition tiles; qc1/qc3 are split into lo/hi 16-kt halves.
        et_parts = {}

        def et_alloc_full(pool, qc):
            t = pool.tile([128, 2, SKT, 512], F16, tag="et", name=f"et_{qc}")
            et_parts[qc] = [(t, 0)]

        def et_alloc_half(pool, qc, half):
            t = pool.tile([128, 2, SKT // 2, 512], F16, tag="eth",
                          name=f"et_{qc}_{half}")
            et_parts.setdefault(qc, []).append((t, half * (SKT // 2)))

        def e_part(qc, kt):
            for t, base in et_parts[qc]:
                if base <= kt < base + (SKT if len(et_parts[qc]) == 1 else SKT // 2):
                    return t, kt - base
            raise KeyError((qc, kt))

        def e_slab(qc, h, kt):
            t, k = e_part(qc, kt)
            return t[:, h, k, :]

        def e_both(qc, kt):
            t, k = e_part(qc, kt)
            return t[:, :, k, :]

        et_alloc_full(et_a, 0)

        # aux psum: sums for qc in columns 0..7 (h*4+qsl), plus the m
        # broadcast matmuls reuse the same pool.
        sums_ps = {}
        sums_started = {}

        def sum_mms(qc, kt):
            """N=1 matmuls accumulating exp row-sums; issue after scores kt+1."""
            aux = sums_ps[qc]
            for h in range(2):
                e = e_slab(qc, h, kt)
                for qsl in range(4):
                    first = not sums_started.get(qc)
                    sums_started[qc] = True
                    last = kt == SKT - 1 and h == 1 and qsl == 3
                    nc.tensor.matmul(
                        aux[:, h * 4 + qsl: h * 4 + qsl + 1],
                        e[:, qsl * 128:(qsl + 1) * 128],
                        ones_col,
                        start=first,
                        stop=last,
                        skip_group_check=not (first or last),
                    )

        # --- projection phase, with qc0 scores/exp interleaved ----------
        sps1 = {}
        for h in range(2):
            sps1[h] = psum_s1.tile([128, 512], F32, tag="sps1", name=f"sps1_{h}")
        sums_ps[0] = psum_aux.tile([128, 512], F32, tag="aux", name="aux0")

        # Scores for qc0 (all chunks) and qc1's lo half are woven between
        # the projection MM groups so each exp has slack before the PE
        # needs its psum tile back (single-wide, one tile per head).
        _SENT = object()
        stepq = []

        def score_steps(qc, kts, qcol):
            """Generator: one (score MM + exp + lagged sums) per next()."""
            for kt in kts:
                for h in range(2):
                    nc.tensor.matmul(
                        sps1[h], kT[:, h, kt * 128:(kt + 1) * 128],
                        qT[:, h, qcol:qcol + 512], start=True, stop=True,
                    )
                    nc.scalar.activation(
                        out=e_slab(qc, h, kt), in_=sps1[h],
                        func=Exp, scale=SCALE, bias=bias_sb,
                    )
                if kt >= 2:
                    sum_mms(qc, kt - 2)
                yield

        def step(n=1):
            for _ in range(n):
                while stepq:
                    if next(stepq[0], _SENT) is _SENT:
                        stepq.pop(0)
                        continue
                    break

        xts = [xt0, xt1]
        for sc in range(SC):
            xt = xts[sc % 2]
            if sc == QC:
                wq_ctx.close()
                et_alloc_half(et_b_lo, 1, 0)
                sums_ps[1] = psum_aux.tile([128, 512], F32, tag="aux",
                                           name="aux1")
            if sc > 0:
                stepq.append(score_steps(0, range((sc - 1) * 4, sc * 4), 0))
            if sc >= QC:
                p = sc - QC
                stepq.append(score_steps(1, range(p * 4, p * 4 + 4), 512))
            # K projection (both heads)
            for h in range(2):
                ps = psum_proj.tile([128, 512], F32, tag="big", name=f"k{sc}{h}")
                for dt in range(DT_P):
                    nc.tensor.matmul(
                        ps,
                        w_sb["wk"][:, dt, h * HD:(h + 1) * HD],
                        xt[:, dt, :],
                        start=(dt == 0),
                        stop=(dt == DT_P - 1),
                    )
                nc.vector.tensor_copy(kT[:, h, sc * 512:(sc + 1) * 512], ps)
                step()
            # Q projection (first 4 chunks only)
            if sc < QC:
                for h in range(2):
                    ps = psum_proj.tile([128, 512], F32, tag="big", name=f"q{sc}{h}")
                    for dt in range(DT_P):
                        nc.tensor.matmul(
                            ps,
                            w_sb["wq"][:, dt, h * HD:(h + 1) * HD],
                            xt[:, dt, :],
                            start=(dt == 0),
                            stop=(dt == DT_P - 1),
                        )
                    nc.vector.tensor_copy(qT[:, h, sc * 512:(sc + 1) * 512], ps)
                    step()
            # V projection (4 s-tiles per chunk)
            for i in range(4):
                vps = psum_v.tile([128, DV], F32, tag="vps", name=f"v{sc}_{i}")
                for dt in range(DT_P):
                    nc.tensor.matmul(
                        vps,
                        xt[:, dt, i * 128:(i + 1) * 128],
                        w_sb["wv"][:, dt, :],
                        start=(dt == 0),
                        stop=(dt == DT_P - 1),
                    )
                nc.vector.tensor_copy(v_sb[:, sc * 4 + i, :], vps)
                step()
            # prefetch chunk sc+2
            if sc + 2 < SC:
                xts[sc % 2] = load_x_chunk(sc + 2)
            # drain this chunk's pending score steps
            step(8)
        # chunk 7's qc0 scores + trailing sums
        stepq.append(score_steps(0, range((SC - 1) * 4, SC * 4), 0))
        step(9)
        for kt in (SKT - 2, SKT - 1):
            sum_mms(0, kt)

        proj_ctx.close()

        # --- attention phase --------------------------------------------
        et_b_hi = ctx.enter_context(tc.tile_pool(name="et_b_hi", bufs=1))
        et_alloc_half(et_b_hi, 1, 1)
        psum_s2 = ctx.enter_context(
            tc.tile_pool(name="psum_s2", bufs=2, space=bass.MemorySpace.PSUM)
        )
        psum_pv = ctx.enter_context(
            tc.tile_pool(name="psum_pv", bufs=2, space=bass.MemorySpace.PSUM)
        )

        def pv_kts(qc, kts, pv):
            for kt in kts:
                for qsl in range(4):
                    first = kt == 0 and qsl % 2 == 0
                    last = kt == SKT - 1 and qsl % 2 == 1
                    t, k = e_part(qc, kt)
                    nc.tensor.matmul(
                        pv[qsl // 2][:, (qsl % 2) * 256:(qsl % 2) * 256 + 256],
                        t[:, 0, k, qsl * 128:(qsl + 1) * 128],
                        v_sb[:, kt, :],
                        start=first,
                        stop=last,
                        skip_group_check=not (first or last),
                    )

        def weave_qc(qc, prev, kt_start=0):
            """2-wide scores+exp for qc's kts [kt_start..32), woven with
            sums(qc) and the whole PV(prev).

            Per-iteration PE order is [pv][sums][scores] so independent work
            precedes the ACT-gated score matmuls; the previous qc's
            m-broadcast + combine issue after iteration 0 (by then the
            m-chain recips/DMA have had a scores-iteration to run).
            """
            if qc not in sums_ps:
                sums_ps[qc] = psum_aux.tile([128, 512], F32, tag="aux",
                                            name=f"aux{qc}")
            if qc == 2:
                et_alloc_full(et_a, 2)
            elif qc == 3:
                et_alloc_half(et_b_lo, 3, 0)
                et_alloc_half(et_b_hi, 3, 1)
            pv = [
                psum_pv.tile([128, 512], F32, tag="pv", name=f"pv{prev}_{b}")
                for b in range(2)
            ]
            n_it = SKT - kt_start
            pv_per = 2 if n_it <= 16 else 1
            for i, kt in enumerate(range(kt_start, SKT)):
                if i == 3:
                    prep_pe(prev)
                    combine_qc(prev)
                if i >= 4:
                    base = (i - 4) * pv_per
                    pv_kts(prev, range(base, min(base + pv_per, SKT - 4)), pv)
                if kt >= 2:
                    sum_mms(qc, kt - 2)
                sps = psum_s2.tile([128, 2, 512], F32, tag="sps2", bufs=2,
                                   name=f"s{qc}_{kt}")
                for h in range(2):
                    nc.tensor.matmul(
                        sps[:, h, :],
                        kT[:, h, kt * 128:(kt + 1) * 128],
                        qT[:, h, qc * 512:(qc + 1) * 512],
                        start=True,
                        stop=True,
                    )
                nc.scalar.activation(
                    out=e_both(qc, kt),
                    in_=sps,
                    func=Exp, scale=SCALE, bias=bias_sb,
                )
            done = min(max(n_it - 4, 0) * pv_per, SKT - 4)
            pv_kts(prev, range(done, SKT), pv)
            for kt in (SKT - 2, SKT - 1):
                sum_mms(qc, kt)
            finish_qc(prev, pv)

        def finish_qc(qc, pv):
            r1s = prep_state[qc]["r1s"]
            for qsl in range(4):
                idx = qc * 4 + qsl
                ot = o_pool.tile([128, DV], F32, tag="ot", name=f"ot{idx}")
                nc.vector.tensor_scalar_mul(
                    ot, pv[qsl // 2][:, (qsl % 2) * 256:(qsl % 2) * 256 + 256],
                    r1s[qsl],
                )
                nc.sync.dma_start(out=out[idx * 128:(idx + 1) * 128, :], in_=ot)

        prep_state = {}

        def prep_ve(qc):
            """m-chain VE part: reciprocals, m column, one transpose DMA."""
            aux = sums_ps[qc]
            m_row = mrow_pool.tile([1, 512], F16, tag="mrow", name=f"mrow{qc}")
            m4 = r_pool.tile([128, 4], F16, tag="m4", name=f"m4_{qc}")
            r1s = []
            for qsl in range(4):
                r1 = r_pool.tile([128, 1], F32, tag="r1", name=f"r1_{qc}_{qsl}")
                nc.vector.reciprocal(r1, aux[:, qsl:qsl + 1])
                r1s.append(r1)
                rs2 = r_pool.tile([128, 1], F32, tag="rs2", name=f"rs2_{qc}_{qsl}")
                nc.vector.reciprocal(rs2, aux[:, 4 + qsl:5 + qsl])
                mt = r_pool.tile([128, 1], F32, tag="mt", name=f"mt_{qc}_{qsl}")
                nc.vector.tensor_mul(mt, aux[:, qsl:qsl + 1], rs2)
                nc.vector.tensor_mul(m4[:, qsl:qsl + 1], mt, lam_sb)
            for qsl in range(4):
                nc.sync.dma_start(
                    out=m_row[0:1, qsl * 128:(qsl + 1) * 128],
                    in_=m4[:, qsl:qsl + 1],
                )
            prep_state[qc] = {"r1s": r1s, "m_row": m_row}

        def prep_pe(qc):
            """m broadcast: K=1 matmul + copy to SBUF fp16."""
            mb_ps = psum_aux.tile([128, 512], F32, tag="aux", name=f"mb_ps{qc}")
            nc.tensor.matmul(mb_ps, ones_row, prep_state[qc]["m_row"],
                             start=True, stop=True)
            m_b = mb_pool.tile([128, 512], F16, tag="mb", name=f"mb{qc}")
            nc.vector.tensor_copy(m_b, mb_ps)
            prep_state[qc]["m_b"] = m_b

        def combine_qc(qc):
            """VE/Pool combine, writing attn in place over the e1 slab."""
            m_b = prep_state[qc]["m_b"]
            for kt in range(SKT):
                u = u_pool.tile([128, 512], F16, tag="u", name=f"u{qc}_{kt}")
                eng = nc.gpsimd if kt % 3 == 2 else nc.vector
                eng.tensor_mul(u, e_slab(qc, 1, kt), m_b)
                nc.vector.tensor_sub(e_slab(qc, 0, kt), e_slab(qc, 0, kt), u)

        prep_ve(0)
        weave_qc(1, prev=0, kt_start=SKT // 2)
        prep_ve(1)
        weave_qc(2, prev=1)
        prep_ve(2)
        weave_qc(3, prev=2)
        prep_ve(3)
        prep_pe(3)
        combine_qc(3)
        # drain: PV for the last qc
        pv3 = [
            psum_pv.tile([128, 512], F32, tag="pv", name=f"pv3_{b}")
            for b in range(2)
        ]
        for g in range(SKT // 2):
            pv_kts(3, (2 * g, 2 * g + 1), pv3)
        finish_qc(3, pv3)

    nc.compile()
    return nc


def _lam(lambda_q1, lambda_q2, lambda_k1, lambda_k2):
    lam_init = 0.8 - 0.6 * math.exp(-0.3 * DEPTH)
    l1 = math.exp(float(np.sum(lambda_q1.astype(np.float64) * lambda_k1.astype(np.float64))))
    l2 = math.exp(float(np.sum(lambda_q2.astype(np.float64) * lambda_k2.astype(np.float64))))
    return l1 + l2 + lam_init


def kernel(x, WQ, WK, WV, lambda_q1, lambda_q2, lambda_k1, lambda_k2):
    if "nc" not in _cache:
        _cache["nc"] = build_nc()
    nc = _cache["nc"]

    lam = np.full((128, 1), _lam(lambda_q1, lambda_q2, lambda_k1, lambda_k2), np.float32)
    wq = np.ascontiguousarray(WQ, dtype=np.float16)
    wk = np.ascontiguousarray(WK, dtype=np.float16)
    wv = np.ascontiguousarray(WV, dtype=np.float16)

    in_maps = []
    for c in range(N_CORES):
        b, qs = c // 2, (c % 2) * SQ
        xb = x[b] if qs == 0 else np.concatenate([x[b, qs:], x[b, :qs]], axis=0)
        xTc = np.ascontiguousarray(xb.T, dtype=np.float16)
        in_maps.append({"xT": xTc, "wq": wq, "wk": wk, "wv": wv, "lam": lam})

    kres = None
    for attempt in range(3):
        try:
            kres = run_bass_kernel_spmd(nc, in_maps, list(range(N_CORES)))
            break
        except (ModuleNotFoundError, ImportError):
            os.environ["BASS_NEVER_TRACE"] = "1"
        except Exception:
            if attempt == 2:
                raise
            time.sleep(5)
    if kres is None:
        kres = run_bass_kernel_spmd(nc, in_maps, list(range(N_CORES)))
    _cache["last_results"] = kres
    res = kres.results

    out = np.empty((B, S, DV), np.float32)
    for c in range(N_CORES):
        b, qs = c // 2, (c % 2) * SQ
        out[b, qs:qs + SQ] = res[c]["out"]
    return out


# revision 59
# speedup vs baseline: 1.1239x; 1.0078x over previous
"""Differential attention kernel for 8 Trainium2 NeuronCores — v3.

Sharding: core c handles batch c//2, query half c%2 (2048 queries x 4096
keys); K/V are computed per-core from a rotated x so keys cover the full
sequence.  291.9us vs the 325.5us two-PV v1 baseline, rel err 7.5e-4 vs
4.5e-3.

Structure vs the v1 two-PV baseline:
- fp16 everywhere instead of bf16 (same cost-model speed, 6x better
  accuracy on the max-rel metric).
- single combined PV: attn = e1 - m_b*e2 with m = lam*s1/s2 per query,
  halving the PV matmul cycles (110us -> 55us of PE); out = r1 * (attn@v).
  exp carries a -2 bias (cancelled exactly by the normalization) to keep
  e2*m inside fp16 range.
- row sums via N=1 matmuls (the cost model charges only the output free
  dim), 8 accumulation chains sharing one PSUM bank (shared start/stop).
- m broadcast to a [128, 512] matrix via DMA sbuf->sbuf transposes of the
  per-qslice [128,1] columns plus a K=1 PE matmul against a ones row.
- scores+exp for qc0 AND the first half of qc1 are woven between the
  projection-phase matmul groups, so the scalar engine's 16.7M-element
  exp workload (the second-busiest engine, ~140us) largely overlaps the
  PE-bound projection phase instead of serializing after it.
- the combine (attn = e1 - u) writes in place over the e1 slab of et,
  saving 16KB/partition of SBUF and any attn-ring stalls.
- PV accumulates 4 q-slices in 2 PSUM banks (2 chains per bank, shared
  start/stop) in a single pass per key tile, woven into the next
  q-chunk's score stream.
"""

import math
import os
import time
from contextlib import ExitStack

import ml_dtypes
import numpy as np

import concourse.bass as bass
from concourse import bacc
import concourse.mybir as mybir
import concourse.tile as tile
from concourse.bass_utils import run_bass_kernel_spmd

B, S, D = 4, 4096, 2048
HD = 128
DV = 256
SQ = S // 2
N_CORES = 8
DEPTH = 12
SCALE = HD ** -0.5
EXP_BIAS = -2.0

DT_P = D // 128   # 16 d-tiles
SKT = S // 128    # 32 key tiles
SC = S // 512     # 8 s-chunks
QC = SQ // 512    # 4 q-chunks
SQT = SQ // 128   # 16 q tiles

F16 = mybir.dt.float16
F32 = mybir.dt.float32

_cache = {}


def build_nc():
    nc = bacc.Bacc("TRN2", target_bir_lowering=False, debug=False)

    xT_d = nc.declare_dram_parameter("xT", [D, S], F16, isOutput=False)
    wq_d = nc.declare_dram_parameter("wq", [D, DV], F16, isOutput=False)
    wk_d = nc.declare_dram_parameter("wk", [D, DV], F16, isOutput=False)
    wv_d = nc.declare_dram_parameter("wv", [D, DV], F16, isOutput=False)
    lam_d = nc.declare_dram_parameter("lam", [128, 1], F32, isOutput=False)
    out_d = nc.declare_dram_parameter("out", [SQ, DV], F32, isOutput=True)

    xT = xT_d.ap()
    out = out_d.ap()
    Exp = mybir.ActivationFunctionType.Exp

    with tile.TileContext(nc) as tc, ExitStack() as ctx:
        singles = ctx.enter_context(tc.tile_pool(name="singles", bufs=1))
        o_pool = ctx.enter_context(tc.tile_pool(name="o", bufs=3))
        r_pool = ctx.enter_context(tc.tile_pool(name="r", bufs=10))
        u_pool = ctx.enter_context(tc.tile_pool(name="u", bufs=3))
        mb_pool = ctx.enter_context(tc.tile_pool(name="mb", bufs=2))
        mrow_pool = ctx.enter_context(tc.tile_pool(name="mrow", bufs=2))
        # et pool A (full 64KB/partition tiles) serves qc0/qc2 as a ring of
        # one; qc1/qc3 are split lo/hi.  Pool B-lo must open before the
        # proj-phase pools (stack allocator) since it outlives them; B-hi
        # opens after they pop.
        et_a = ctx.enter_context(tc.tile_pool(name="et_a", bufs=1))
        et_b_lo = ctx.enter_context(tc.tile_pool(name="et_b_lo", bufs=1))

        psum_aux = ctx.enter_context(
            tc.tile_pool(name="psum_aux", bufs=2, space=bass.MemorySpace.PSUM)
        )

        # --- persistent SBUF tensors ------------------------------------
        kT = singles.tile([128, 2, S], F16, tag="kT")        # [dh, head, key]
        qT = singles.tile([128, 2, SQ], F16, tag="qT")       # [dh, head, q]
        v_sb = singles.tile([128, SKT, DV], F16, tag="v")    # [krow, ktile, dv]
        lam_sb = singles.tile([128, 1], F32, tag="lam")
        ones_col = singles.tile([128, 1], F16, tag="ones_col")
        ones_row = singles.tile([1, 128], F16, tag="ones_row")
        jt = singles.tile([128, 512], F16, tag="junk")

        bias_sb = singles.tile([128, 1], F32, tag="bias_sb")
        nc.sync.dma_start(out=lam_sb, in_=lam_d.ap())
        nc.vector.memset(ones_col, 1.0)
        nc.vector.memset(ones_row, 1.0)
        nc.vector.memset(jt, 0.0)
        nc.vector.memset(bias_sb, EXP_BIAS)

        proj_ctx = ExitStack()
        w_pool = proj_ctx.enter_context(tc.tile_pool(name="w", bufs=1))
        x_pool = proj_ctx.enter_context(tc.tile_pool(name="x", bufs=2))
        psum_proj = proj_ctx.enter_context(
            tc.tile_pool(name="psum_proj", bufs=2, space=bass.MemorySpace.PSUM)
        )
        psum_v = proj_ctx.enter_context(
            tc.tile_pool(name="psum_v", bufs=2, space=bass.MemorySpace.PSUM)
        )
        psum_s1 = proj_ctx.enter_context(
            tc.tile_pool(name="psum_s1", bufs=2, space=bass.MemorySpace.PSUM)
        )

        # x chunks: [128, dt, 512] layout; chunk 0 is split into quarters
        # so the first K-projection matmuls can start as soon as wk plus a
        # quarter of x0 have landed.
        def load_x_chunk(sc, split=1):
            xt = x_pool.tile([128, DT_P, 512], F16, tag="xt", name=f"x{sc}")
            q = DT_P // split
            for i in range(split):
                nc.sync.dma_start(
                    out=xt[:, i * q:(i + 1) * q, :],
                    in_=xT[i * q * 128:(i + 1) * q * 128,
                           sc * 512:(sc + 1) * 512].rearrange(
                        "(t p) v -> p t v", p=128
                    ),
                )
            return xt

        # DMA order: wk, x0 (split), wv, x1, wq — matching first use.  wq
        # lives in its own pool, closed after the Q projection finishes so
        # qc1's lo et tile can take its SBUF.
        wq_ctx = ExitStack()
        wq_pool = wq_ctx.enter_context(tc.tile_pool(name="wqp", bufs=1))
        w_sb = {}

        def load_w(name, dram, pool, split=1):
            t = pool.tile([128, DT_P, DV], F16, tag=f"w_{name}",
                          name=f"w_{name}")
            w_sb[name] = t
            q = DT_P // split
            for i in range(split):
                nc.sync.dma_start(
                    out=t[:, i * q:(i + 1) * q, :],
                    in_=dram.ap()[i * q * 128:(i + 1) * q * 128, :].rearrange(
                        "(t p) v -> p t v", p=128
                    ),
                )

        load_w("wk", wk_d, w_pool, split=2)
        xt0 = load_x_chunk(0, split=4)
        load_w("wv", wv_d, w_pool)
        xt1 = load_x_chunk(1)
        load_w("wq", wq_d, wq_pool)

        # PE warm-up: junk matmuls cover the initial DMA wait and the
        # p-state ramp; a junk exp pulls the ACT table load off the
        # critical path.
        jps = psum_proj.tile([128, 512], F32, tag="big", name="jps")
        for _ in range(12):
            nc.tensor.matmul(jps, jt[:, 0:128], jt, start=True, stop=True)
        jet = r_pool.tile([128, 1], F16, tag="jet", name="jet")
        nc.scalar.activation(out=jet, in_=jt[:, 0:1], func=Exp, scale=SCALE)

        # et tensors: [128, head, kts, 512] fp16.  qc0/qc2 are single
        # 64KB/partition tiles; qc1/qc3 are split into lo/hi 16-kt halves.
        et_parts = {}

        def et_alloc_full(pool, qc):
            t = pool.tile([128, 2, SKT, 512], F16, tag="et", name=f"et_{qc}")
            et_parts[qc] = [(t, 0)]

        def et_alloc_half(pool, qc, half):
            t = pool.tile([128, 2, SKT // 2, 512], F16, tag="eth",
                          name=f"et_{qc}_{half}")
            et_parts.setdefault(qc, []).append((t, half * (SKT // 2)))

        def e_part(qc, kt):
            for t, base in et_parts[qc]:
                if base <= kt < base + (SKT if len(et_parts[qc]) == 1 else SKT // 2):
                    return t, kt - base
            raise KeyError((qc, kt))

        def e_slab(qc, h, kt):
            t, k = e_part(qc, kt)
            return t[:, h, k, :]

        def e_both(qc, kt):
            t, k = e_part(qc, kt)
            return t[:, :, k, :]

        et_alloc_full(et_a, 0)

        # aux psum: sums for qc in columns 0..7 (h*4+qsl), plus the m
        # broadcast matmuls reuse the same pool.
        sums_ps = {}
        sums_started = {}

        def sum_mms(qc, kt):
            """N=1 matmuls accumulating exp row-sums; issue after scores kt+1."""
            aux = sums_ps[qc]
            for h in range(2):
                e = e_slab(qc, h, kt)
                for qsl in range(4):
                    first = not sums_started.get(qc)
                    sums_started[qc] = True
                    last = kt == SKT - 1 and h == 1 and qsl == 3
                    nc.tensor.matmul(
                        aux[:, h * 4 + qsl: h * 4 + qsl + 1],
                        e[:, qsl * 128:(qsl + 1) * 128],
                        ones_col,
                        start=first,
                        stop=last,
                        skip_group_check=not (first or last),
                    )

        # --- projection phase, with qc0 scores/exp interleaved ----------
        sps1 = {}
        for h in range(2):
            sps1[h] = psum_s1.tile([128, 512], F32, tag="sps1", name=f"sps1_{h}")
        sums_ps[0] = psum_aux.tile([128, 512], F32, tag="aux", name="aux0")

        # Scores for qc0 (all chunks) and qc1's lo half are woven between
        # the projection MM groups so each exp has slack before the PE
        # needs its psum tile back (single-wide, one tile per head).
        _SENT = object()
        stepq = []

        def score_steps(qc, kts, qcol):
            """Generator: one (score MM + exp + lagged sums) per next()."""
            for kt in kts:
                for h in range(2):
                    nc.tensor.matmul(
                        sps1[h], kT[:, h, kt * 128:(kt + 1) * 128],
                        qT[:, h, qcol:qcol + 512], start=True, stop=True,
                    )
                    nc.scalar.activation(
                        out=e_slab(qc, h, kt), in_=sps1[h],
                        func=Exp, scale=SCALE, bias=bias_sb,
                    )
                if kt >= 2:
                    sum_mms(qc, kt - 2)
                yield

        def step(n=1):
            for _ in range(n):
                while stepq:
                    if next(stepq[0], _SENT) is _SENT:
                        stepq.pop(0)
                        continue
                    break

        xts = [xt0, xt1]
        for sc in range(SC):
            xt = xts[sc % 2]
            if sc == QC:
                wq_ctx.close()
                et_alloc_half(et_b_lo, 1, 0)
                sums_ps[1] = psum_aux.tile([128, 512], F32, tag="aux",
                                           name="aux1")
            if sc > 0:
                stepq.append(score_steps(0, range((sc - 1) * 4, sc * 4), 0))
            if sc >= QC:
                p = sc - QC
                stepq.append(score_steps(1, range(p * 4, p * 4 + 4), 512))
            # K projection (both heads)
            for h in range(2):
                ps = psum_proj.tile([128, 512], F32, tag="big", name=f"k{sc}{h}")
                for dt in range(DT_P):
                    nc.tensor.matmul(
                        ps,
                        w_sb["wk"][:, dt, h * HD:(h + 1) * HD],
                        xt[:, dt, :],
                        start=(dt == 0),
                        stop=(dt == DT_P - 1),
                    )
                nc.vector.tensor_copy(kT[:, h, sc * 512:(sc + 1) * 512], ps)
                step()
            # Q projection (first 4 chunks only)
            if sc < QC:
                for h in range(2):
                    ps = psum_proj.tile([128, 512], F32, tag="big", name=f"q{sc}{h}")
                    for dt in range(DT_P):
                        nc.tensor.matmul(
                            ps,
                            w_sb["wq"][:, dt, h * HD:(h + 1) * HD],
                            xt[:, dt, :],
                            start=(dt == 0),
                            stop=(dt == DT_P - 1),
                        )
                    nc.vector.tensor_copy(qT[:, h, sc * 512:(sc + 1) * 512], ps)
                    step()
            # V projection (4 s-tiles per chunk)
            for i in range(4):
                vps = psum_v.tile([128, DV], F32, tag="vps", name=f"v{sc}_{i}")
                for dt in range(DT_P):
                    nc.tensor.matmul(
                        vps,
                        xt[:, dt, i * 128:(i + 1) * 128],
                        w_sb["wv"][:, dt, :],
                        start=(dt == 0),
                        stop=(dt == DT_P - 1),
                    )
                nc.vector.tensor_copy(v_sb[:, sc * 4 + i, :], vps)
                step()
            # prefetch chunk sc+2
            if sc + 2 < SC:
                xts[sc % 2] = load_x_chunk(sc + 2)
            # drain this chunk's pending score steps
            step(8)
        # chunk 7's qc0 scores + trailing sums
        stepq.append(score_steps(0, range((SC - 1) * 4, SC * 4), 0))
        step(9)
        for kt in (SKT - 2, SKT - 1):
            sum_mms(0, kt)

        proj_ctx.close()

        # --- attention phase --------------------------------------------
        et_b_hi = ctx.enter_context(tc.tile_pool(name="et_b_hi", bufs=1))
        et_alloc_half(et_b_hi, 1, 1)
        psum_s2 = ctx.enter_context(
            tc.tile_pool(name="psum_s2", bufs=2, space=bass.MemorySpace.PSUM)
        )
        psum_pv = ctx.enter_context(
            tc.tile_pool(name="psum_pv", bufs=2, space=bass.MemorySpace.PSUM)
        )

        def pv_kts(qc, kts, pv):
            for kt in kts:
                for qsl in range(4):
                    first = kt == 0 and qsl % 2 == 0
                    last = kt == SKT - 1 and qsl % 2 == 1
                    t, k = e_part(qc, kt)
                    nc.tensor.matmul(
                        pv[qsl // 2][:, (qsl % 2) * 256:(qsl % 2) * 256 + 256],
                        t[:, 0, k, qsl * 128:(qsl + 1) * 128],
                        v_sb[:, kt, :],
                        start=first,
                        stop=last,
                        skip_group_check=not (first or last),
                    )

        def ensure_qc(qc):
            if qc not in sums_ps:
                sums_ps[qc] = psum_aux.tile([128, 512], F32, tag="aux",
                                            name=f"aux{qc}")
            if qc not in et_parts:
                if qc == 2:
                    et_alloc_full(et_a, 2)
                elif qc == 3:
                    et_alloc_half(et_b_lo, 3, 0)
                    et_alloc_half(et_b_hi, 3, 1)

        def score_kt(qc, kt):
            """One kt of 2-wide scores + exp + lagged sums for qc."""
            sps = psum_s2.tile([128, 2, 512], F32, tag="sps2", bufs=2,
                               name=f"s{qc}_{kt}")
            for h in range(2):
                nc.tensor.matmul(
                    sps[:, h, :],
                    kT[:, h, kt * 128:(kt + 1) * 128],
                    qT[:, h, qc * 512:(qc + 1) * 512],
                    start=True,
                    stop=True,
                )
            nc.scalar.activation(
                out=e_both(qc, kt),
                in_=sps,
                func=Exp, scale=SCALE, bias=bias_sb,
            )
            if kt >= 2:
                sum_mms(qc, kt - 2)

        def weave_qc(qc, prev, kt_start=0, next_qc=None, next_n=0):
            """2-wide scores+exp for qc's kts [kt_start..32), woven with
            sums(qc) and the whole PV(prev); optionally followed by a
            head-start on the NEXT qc's first kts to soak idle ACT time.

            Per-iteration PE order is [pv][sums][scores] so independent work
            precedes the ACT-gated score matmuls; the previous qc's
            m-broadcast + combine issue after iteration 3 (by then the
            m-chain recips/DMA have had a few scores-iterations to run).
            """
            ensure_qc(qc)
            pv = [
                psum_pv.tile([128, 512], F32, tag="pv", name=f"pv{prev}_{b}")
                for b in range(2)
            ]
            n_it = SKT - kt_start
            pv_per = 2 if n_it <= 16 else 1
            for i, kt in enumerate(range(kt_start, SKT)):
                if i == 3:
                    prep_pe(prev)
                    combine_qc(prev)
                if i >= 4:
                    base = (i - 4) * pv_per
                    pv_kts(prev, range(base, min(base + pv_per, SKT - 4)), pv)
                score_kt(qc, kt)
            done = min(max(n_it - 4, 0) * pv_per, SKT - 4)
            pv_kts(prev, range(done, SKT), pv)
            for kt in (SKT - 2, SKT - 1):
                sum_mms(qc, kt)
            if next_qc is not None:
                ensure_qc(next_qc)
                for kt in range(next_n):
                    score_kt(next_qc, kt)
            finish_qc(prev, pv)

        def finish_qc(qc, pv, use_act=False):
            """Post-scale by 1/s1 and store.  For the last qc (drain, ACT
            idle) odd q-slices go through the scalar engine (Identity with
            per-partition scale) so the tail runs on two engines."""
            r1s = prep_state[qc]["r1s"]
            for qsl in range(4):
                idx = qc * 4 + qsl
                ot = o_pool.tile([128, DV], F32, tag="ot", name=f"ot{idx}")
                if use_act and qsl % 2:
                    nc.scalar.activation(
                        out=ot,
                        in_=pv[qsl // 2][:, (qsl % 2) * 256:(qsl % 2) * 256 + 256],
                        func=mybir.ActivationFunctionType.Identity,
                        scale=r1s[qsl],
                    )
                    nc.sync.dma_start(out=out[idx * 128:(idx + 1) * 128, :], in_=ot)
                    continue
                nc.vector.tensor_scalar_mul(
                    ot, pv[qsl // 2][:, (qsl % 2) * 256:(qsl % 2) * 256 + 256],
                    r1s[qsl],
                )
                nc.sync.dma_start(out=out[idx * 128:(idx + 1) * 128, :], in_=ot)

        prep_state = {}

        def prep_ve(qc):
            """m-chain VE part: reciprocals, m column, one transpose DMA."""
            aux = sums_ps[qc]
            m_row = mrow_pool.tile([1, 512], F16, tag="mrow", name=f"mrow{qc}")
            m4 = r_pool.tile([128, 4], F16, tag="m4", name=f"m4_{qc}")
            r1s = []
            for qsl in range(4):
                r1 = r_pool.tile([128, 1], F32, tag="r1", name=f"r1_{qc}_{qsl}")
                nc.vector.reciprocal(r1, aux[:, qsl:qsl + 1])
                r1s.append(r1)
                rs2 = r_pool.tile([128, 1], F32, tag="rs2", name=f"rs2_{qc}_{qsl}")
                nc.vector.reciprocal(rs2, aux[:, 4 + qsl:5 + qsl])
                mt = r_pool.tile([128, 1], F32, tag="mt", name=f"mt_{qc}_{qsl}")
                nc.vector.tensor_mul(mt, aux[:, qsl:qsl + 1], rs2)
                nc.vector.tensor_mul(m4[:, qsl:qsl + 1], mt, lam_sb)
            for qsl in range(4):
                nc.sync.dma_start(
                    out=m_row[0:1, qsl * 128:(qsl + 1) * 128],
                    in_=m4[:, qsl:qsl + 1],
                )
            prep_state[qc] = {"r1s": r1s, "m_row": m_row}

        def prep_pe(qc):
            """m broadcast: K=1 matmul + copy to SBUF fp16."""
            mb_ps = psum_aux.tile([128, 512], F32, tag="aux", name=f"mb_ps{qc}")
            nc.tensor.matmul(mb_ps, ones_row, prep_state[qc]["m_row"],
                             start=True, stop=True)
            m_b = mb_pool.tile([128, 512], F16, tag="mb", name=f"mb{qc}")
            nc.vector.tensor_copy(m_b, mb_ps)
            prep_state[qc]["m_b"] = m_b

        def combine_qc(qc):
            """VE/Pool combine, writing attn in place over the e1 slab."""
            m_b = prep_state[qc]["m_b"]
            for kt in range(SKT):
                u = u_pool.tile([128, 512], F16, tag="u", name=f"u{qc}_{kt}")
                eng = nc.gpsimd if kt % 3 == 2 else nc.vector
                eng.tensor_mul(u, e_slab(qc, 1, kt), m_b)
                sub_eng = nc.gpsimd if kt % 8 == 5 else nc.vector
                sub_eng.tensor_sub(e_slab(qc, 0, kt), e_slab(qc, 0, kt), u)

        prep_ve(0)
        weave_qc(1, prev=0, kt_start=SKT // 2, next_qc=2, next_n=8)
        prep_ve(1)
        weave_qc(2, prev=1, kt_start=8)
        prep_ve(2)
        weave_qc(3, prev=2)
        prep_ve(3)
        prep_pe(3)
        combine_qc(3)
        # drain: PV for the last qc
        pv3 = [
            psum_pv.tile([128, 512], F32, tag="pv", name=f"pv3_{b}")
            for b in range(2)
        ]
        for g in range(SKT // 2):
            pv_kts(3, (2 * g, 2 * g + 1), pv3)
        finish_qc(3, pv3)

    nc.compile()
    return nc


def _lam(lambda_q1, lambda_q2, lambda_k1, lambda_k2):
    lam_init = 0.8 - 0.6 * math.exp(-0.3 * DEPTH)
    l1 = math.exp(float(np.sum(lambda_q1.astype(np.float64) * lambda_k1.astype(np.float64))))
    l2 = math.exp(float(np.sum(lambda_q2.astype(np.float64) * lambda_k2.astype(np.float64))))
    return l1 + l2 + lam_init


def kernel(x, WQ, WK, WV, lambda_q1, lambda_q2, lambda_k1, lambda_k2):
    if "nc" not in _cache:
        _cache["nc"] = build_nc()
    nc = _cache["nc"]

    lam = np.full((128, 1), _lam(lambda_q1, lambda_q2, lambda_k1, lambda_k2), np.float32)
    wq = np.ascontiguousarray(WQ, dtype=np.float16)
    wk = np.ascontiguousarray(WK, dtype=np.float16)
    wv = np.ascontiguousarray(WV, dtype=np.float16)

    in_maps = []
    for c in range(N_CORES):
        b, qs = c // 2, (c % 2) * SQ
        xb = x[b] if qs == 0 else np.concatenate([x[b, qs:], x[b, :qs]], axis=0)
        xTc = np.ascontiguousarray(xb.T, dtype=np.float16)
        in_maps.append({"xT": xTc, "wq": wq, "wk": wk, "wv": wv, "lam": lam})

    kres = None
    for attempt in range(3):
        try:
            kres = run_bass_kernel_spmd(nc, in_maps, list(range(N_CORES)))
            break
        except (ModuleNotFoundError, ImportError):
            os.environ["BASS_NEVER_TRACE"] = "1"
        except Exception:
            if attempt == 2:
                raise
            time.sleep(5)
    if kres is None:
        kres = run_bass_kernel_spmd(nc, in_maps, list(range(N_CORES)))
    _cache["last_results"] = kres
    res = kres.results

    out = np.empty((B, S, DV), np.float32)
    for c in range(N_CORES):
        b, qs = c // 2, (c % 2) * SQ
        out[b, qs:qs + SQ] = res[c]["out"]
    return out
